# revision 31
# baseline (speedup 1.0000x reference)
"""DiT MoE router kernel for Trainium2 (8 NeuronCores, SPMD).

Computes, for x [B,S,H] and gate weight W [E,H]:
  logits = x @ W.T              (gate GEMM, E=16)
  probs  = softmax(logits)
  top2 probs (renormalized) + expert indices + load-balancing aux loss.

Sharding: tokens (B*S) split into 8 contiguous slabs of 2048, one per core;
W replicated. Each core returns its top-2 probs/indices plus per-partition
count/prob-sum accumulators; the host reduces the [E]-sized stats and forms
the aux-loss scalar (the cross-core reduction is 8*2*16 floats, so an
on-device collective would only add latency).

Device GEMM runs in float32r (FP22 multiplies, fp32 accumulate). Tokens whose
top-3 logit margins are below a safety threshold (~1e-3, vs FP22 logit error
~2e-4) are recomputed exactly on the host in float64 — a few dozen tokens out
of 16K — so returned indices match a full-fp32 reference everywhere the
reference itself is numerically well-defined.
"""

import numpy as np

import concourse.bass as bass
import concourse.mybir as mybir
import concourse.tile as tile
from concourse import bacc

NCORES = 8
B, S, H, E, TOPK = 4, 4096, 4096, 16, 2
T = (B * S) // NCORES  # tokens per core (2048)
P = 128                # partitions
NT = T // P            # token tiles per core (16)
HC = H // P            # contraction chunks (32)
G = 256                # tokens per GEMM group (matmul free dim)
SUB = G // P           # token tiles per group (2)
NG = T // G            # groups per core (8)

F32 = mybir.dt.float32
F32R = mybir.dt.float32r
U32 = mybir.dt.uint32

_NC_CACHE = {}


def _build_nc():
    from contextlib import ExitStack

    # Bacc (not plain Bass): its compile() pass splits multi-wait sync
    # conditions, which walrus requires (max 1 wait per instruction).
    nc = bacc.Bacc(None)
    x = nc.dram_tensor("x", [T, H], F32, kind="ExternalInput")
    # W^T pre-interleaved and FP22-pre-truncated on the host:
    # w_t[p, c*E + e] = trunc22(W[e, c*128 + p])
    w_t = nc.dram_tensor("wt", [P, HC * E], F32R, kind="ExternalInput")
    ident_in = nc.dram_tensor("ident", [P, P], F32, kind="ExternalInput")
    o_probs = nc.dram_tensor("o_probs", [T, TOPK], F32, kind="ExternalOutput")
    o_idx = nc.dram_tensor("o_idx", [T, TOPK], U32, kind="ExternalOutput")
    o_m4 = nc.dram_tensor("o_m4", [T, 4], F32, kind="ExternalOutput")
    o_stats = nc.dram_tensor("o_stats", [P, 2 * E], F32, kind="ExternalOutput")

    with tile.TileContext(nc) as tc, ExitStack() as ctx:
        const = ctx.enter_context(tc.tile_pool(name="const", bufs=1))
        xpool = ctx.enter_context(tc.tile_pool(name="xpool", bufs=4))
        xtpool = ctx.enter_context(tc.tile_pool(name="xtpool", bufs=6))
        ps_t = ctx.enter_context(tc.tile_pool(name="ps_t", bufs=4, space="PSUM"))
        ps_l = ctx.enter_context(tc.tile_pool(name="ps_l", bufs=2, space="PSUM"))
        ps_lt = ctx.enter_context(tc.tile_pool(name="ps_lt", bufs=2, space="PSUM"))
        small = ctx.enter_context(tc.tile_pool(name="small", bufs=4))
        accp = ctx.enter_context(tc.tile_pool(name="accp", bufs=1))

        # Constants arrive via SWDGE so the HWDGE descriptor generator is
        # dedicated to the x stream (desc-gen, not bytes, is the DMA limit).
        ident = const.tile([P, P], F32)
        nc.gpsimd.dma_start(out=ident[:, :], in_=ident_in[:, :])
        wt = const.tile([P, HC * E], F32R)
        nc.gpsimd.dma_start(out=wt[:, :], in_=w_t[:, :])

        # Token mapping: tile tt, partition p <-> token t = p*NT + tt.
        # Partition p's SUB group-tokens are then adjacent in DRAM, so one
        # DMA per group needs only 128 large (SUB*16KB) descriptors, and the
        # staged output DMAs are contiguous per partition too.
        def load_group(g):
            x_in = xpool.tile([P, SUB * H], F32, tag="x", name=f"x_g{g}")
            nc.sync.dma_start(
                out=x_in[:, :],
                in_=bass.AP(x, g * SUB * H, [[NT * H, P], [1, SUB * H]]),
            )
            return x_in

        count_acc = accp.tile([P, E], F32)
        prob_acc = accp.tile([P, E], F32)
        nc.vector.memset(count_acc[:, :], 0.0)
        nc.vector.memset(prob_acc[:, :], 0.0)

        st_probs = accp.tile([P, NT * TOPK], F32)
        st_idx = accp.tile([P, NT * TOPK], U32)
        st_m4 = accp.tile([P, NT * 4], F32)

        for g in range(NG):
            x_in = load_group(g)

            logits_ps = ps_l.tile([E, G], F32, tag="logits", name=f"lp{g}")
            for c in range(HC):
                tr_ps = ps_t.tile([P, G], F32, tag="tr", name=f"tr{g}_{c}")
                for s in range(SUB):
                    nc.tensor.transpose(
                        tr_ps[:, s * P:(s + 1) * P],
                        x_in[:, s * H + c * P:s * H + (c + 1) * P],
                        ident[:, :],
                    )
                xt_sb = xtpool.tile([P, G], F32R, tag="xt", name=f"xt{g}_{c}")
                if c % 2 == 0:
                    nc.scalar.copy(xt_sb[:, :], tr_ps[:, :])
                else:
                    nc.vector.tensor_copy(xt_sb[:, :], tr_ps[:, :])
                nc.tensor.matmul(
                    logits_ps[:, :],
                    wt[:, c * E:(c + 1) * E],
                    xt_sb[:, :],
                    start=(c == 0),
                    stop=(c == HC - 1),
                )

            logits_sb = small.tile([E, G], F32, tag="lsb", name=f"lsb{g}")
            nc.scalar.copy(logits_sb[:, :], logits_ps[:, :])

            for s in range(SUB):
                tt = g * SUB + s
                lt_ps = ps_lt.tile([P, E], F32, tag="lt_ps", name=f"ltp{tt}")
                nc.tensor.transpose(
                    lt_ps[:, :], logits_sb[:, s * P:(s + 1) * P], ident[:E, :E]
                )
                lt = small.tile([P, E], F32, tag="lt", name=f"lt{tt}")
                nc.vector.tensor_copy(lt[:, :], lt_ps[:, :])

                maxs = small.tile([P, 8], F32, tag="maxs", name=f"mx{tt}")
                nc.vector.max(out=maxs[:, :], in_=lt[:, :])
                idx8 = small.tile([P, 8], U32, tag="idx8", name=f"ix{tt}")
                nc.vector.max_index(out=idx8[:, :], in_max=maxs[:, :], in_values=lt[:, :])

                nc.vector.tensor_copy(st_m4[:, tt * 4:(tt + 1) * 4], maxs[:, 0:4])
                nc.vector.tensor_copy(st_idx[:, tt * TOPK:(tt + 1) * TOPK], idx8[:, 0:TOPK])

                neg1 = small.tile([P, 1], F32, tag="neg1", name=f"n1{tt}")
                nc.vector.tensor_scalar_mul(neg1[:, :], maxs[:, 0:1], -1.0)

                exp_t = small.tile([P, E], F32, tag="expt", name=f"ex{tt}")
                denom = small.tile([P, 1], F32, tag="denom", name=f"dn{tt}")
                nc.scalar.activation(
                    exp_t[:, :], lt[:, :], mybir.ActivationFunctionType.Exp,
                    bias=neg1[:, :], scale=1.0, accum_out=denom[:, :],
                )
                rden = small.tile([P, 1], F32, tag="rden", name=f"rd{tt}")
                nc.vector.reciprocal(rden[:, :], denom[:, :])
                probs_t = small.tile([P, E], F32, tag="probst", name=f"pb{tt}")
                nc.vector.tensor_scalar_mul(probs_t[:, :], exp_t[:, :], rden[:, :])
                nc.vector.tensor_add(prob_acc[:, :], prob_acc[:, :], probs_t[:, :])

                mask = small.tile([P, E], F32, tag="mask", name=f"mk{tt}")
                nc.vector.tensor_scalar(
                    mask[:, :], lt[:, :], maxs[:, 1:2], None, op0=mybir.AluOpType.is_ge
                )
                nc.vector.tensor_add(count_acc[:, :], count_acc[:, :], mask[:, :])

                # renormalized top-2 probs: p1 = 1/(1+e2), p2 = e2/(1+e2)
                e2 = small.tile([P, 1], F32, tag="e2", name=f"e2{tt}")
                nc.scalar.activation(
                    e2[:, :], maxs[:, 1:2], mybir.ActivationFunctionType.Exp,
                    bias=neg1[:, :], scale=1.0,
                )
                dsum = small.tile([P, 1], F32, tag="dsum", name=f"ds{tt}")
                nc.vector.tensor_scalar_add(dsum[:, :], e2[:, :], 1.0)
                rsum = small.tile([P, 1], F32, tag="rsum", name=f"rs{tt}")
                nc.vector.reciprocal(rsum[:, :], dsum[:, :])
                nc.vector.tensor_copy(st_probs[:, tt * TOPK:tt * TOPK + 1], rsum[:, :])
                nc.vector.tensor_mul(
                    st_probs[:, tt * TOPK + 1:tt * TOPK + 2], e2[:, :], rsum[:, :]
                )

                # flush staged outputs every 4 tiles, via SWDGE (GpSimd is
                # otherwise idle) so SP/ACT stay on the critical path work
                if tt % 4 == 3:
                    lo = tt - 3
                    op = o_probs[:, :].rearrange("(p tt) k -> p (tt k)", p=P)
                    oi = o_idx[:, :].rearrange("(p tt) k -> p (tt k)", p=P)
                    om = o_m4[:, :].rearrange("(p tt) k -> p (tt k)", p=P)
                    nc.gpsimd.dma_start(
                        out=op[:, lo * TOPK:(tt + 1) * TOPK],
                        in_=st_probs[:, lo * TOPK:(tt + 1) * TOPK],
                    )
                    nc.gpsimd.dma_start(
                        out=oi[:, lo * TOPK:(tt + 1) * TOPK],
                        in_=st_idx[:, lo * TOPK:(tt + 1) * TOPK],
                    )
                    nc.gpsimd.dma_start(
                        out=om[:, lo * 4:(tt + 1) * 4],
                        in_=st_m4[:, lo * 4:(tt + 1) * 4],
                    )

        nc.gpsimd.dma_start(out=o_stats[:, 0:E], in_=count_acc[:, :])
        nc.gpsimd.dma_start(out=o_stats[:, E:2 * E], in_=prob_acc[:, :])

    nc.finalize()
    return nc


def get_nc():
    if "nc" not in _NC_CACHE:
        _NC_CACHE["nc"] = _build_nc()
    return _NC_CACHE["nc"]


def make_wt_host(Wf):
    """W [E,H] f32 -> [P, HC*E] with wt[p, c*E+e] = trunc22(W[e, c*128+p])."""
    Wt = np.ascontiguousarray(Wf, dtype=np.float32).copy()
    Wt.view(np.uint32)[...] &= np.uint32(0xFFFFFC00)  # FP22 truncation
    return np.ascontiguousarray(
        Wt.reshape(E, HC, P).transpose(2, 1, 0).reshape(P, HC * E)
    )


def _host_postprocess(xt, Wf, probs, idx, m4, counts, psums):
    """Fix tokens whose FP22 top-3 margins are within noise of a tie."""
    thr = 2e-3
    risk = ((m4[:, 0] - m4[:, 1]) < thr) | ((m4[:, 1] - m4[:, 2]) < thr)
    ridx = np.nonzero(risk)[0]
    if ridx.size:
        lg = xt[ridx].astype(np.float64) @ Wf.astype(np.float64).T  # [n, E]
        order = np.argsort(-lg, axis=1, kind="stable")[:, :TOPK].astype(np.int32)
        lsel = np.take_along_axis(lg, order, 1)
        ex = np.exp(lsel - lsel[:, :1])
        pn = ex / ex.sum(1, keepdims=True)
        np.add.at(counts, idx[ridx].reshape(-1), -1.0)
        np.add.at(counts, order.reshape(-1), 1.0)
        idx[ridx] = order
        probs[ridx] = pn.astype(np.float32)
    aux = E * np.sum((counts / B) * (psums / (B * S)))
    return probs, idx, np.float32(aux)


def kernel(x, W):
    from concourse.bass_utils import run_bass_kernel_spmd

    xt = np.ascontiguousarray(np.asarray(x, dtype=np.float32).reshape(B * S, H))
    Wf = np.ascontiguousarray(np.asarray(W, dtype=np.float32))

    nc = get_nc()
    wt_host = make_wt_host(Wf)
    ident_host = np.eye(P, dtype=np.float32)
    in_maps = [
        {"x": xt[i * T:(i + 1) * T], "wt": wt_host, "ident": ident_host}
        for i in range(NCORES)
    ]
    res = run_bass_kernel_spmd(nc, in_maps, core_ids=list(range(NCORES)))

    probs = np.concatenate([r["o_probs"] for r in res.results], axis=0)
    idx = np.concatenate([r["o_idx"] for r in res.results], axis=0).astype(np.int32)
    m4 = np.concatenate([r["o_m4"] for r in res.results], axis=0)
    stats = np.stack([r["o_stats"] for r in res.results])  # [8, P, 2E]
    counts = stats[:, :, :E].sum(axis=(0, 1)).astype(np.float64)
    psums = stats[:, :, E:].sum(axis=(0, 1)).astype(np.float64)

    probs, idx, aux = _host_postprocess(xt, Wf, probs, idx, m4, counts, psums)
    return (
        probs.reshape(B, S, TOPK),
        idx.reshape(B, S, TOPK),
        aux,
    )


# revision 39
# speedup vs baseline: 1.0200x; 1.0200x over previous
"""DiT MoE router kernel for Trainium2 (8 NeuronCores, SPMD).

Computes, for x [B,S,H] and gate weight W [E,H]:
  logits = x @ W.T              (gate GEMM, E=16)
  probs  = softmax(logits)
  top2 probs (renormalized) + expert indices + load-balancing aux loss.

Sharding: tokens (B*S) split into 8 contiguous slabs of 2048, one per core;
W replicated. Each core returns its top-2 probs/indices plus per-partition
count/prob-sum accumulators; the host reduces the [E]-sized stats and forms
the aux-loss scalar (the cross-core reduction is 8*2*16 floats, so an
on-device collective would only add latency).

Device GEMM runs in float32r (FP22 multiplies, fp32 accumulate). Tokens whose
top-3 logit margins are below a safety threshold (~1e-3, vs FP22 logit error
~2e-4) are recomputed exactly on the host in float64 — a few dozen tokens out
of 16K — so returned indices match a full-fp32 reference everywhere the
reference itself is numerically well-defined.
"""

import numpy as np

import concourse.bass as bass
import concourse.mybir as mybir
import concourse.tile as tile
from concourse import bacc

NCORES = 8
B, S, H, E, TOPK = 4, 4096, 4096, 16, 2
T = (B * S) // NCORES  # tokens per core (2048)
P = 128                # partitions
NT = T // P            # token tiles per core (16)
HC = H // P            # contraction chunks (32)
G = 256                # tokens per GEMM group (matmul free dim)
SUB = G // P           # token tiles per group (2)
NG = T // G            # groups per core (8)

F32 = mybir.dt.float32
F32R = mybir.dt.float32r
U32 = mybir.dt.uint32

_NC_CACHE = {}


def _build_nc():
    from contextlib import ExitStack

    # Bacc (not plain Bass): its compile() pass splits multi-wait sync
    # conditions, which walrus requires (max 1 wait per instruction).
    nc = bacc.Bacc(None)
    # x declared float32r end-to-end: the f32r transposes consume it, and the
    # BIR verifier requires f32r consumers to see f32r-typed producers.
    # Bytes are plain f32; the PE read path truncates to FP22.
    x = nc.dram_tensor("x", [T, H], F32R, kind="ExternalInput")
    # W^T pre-interleaved and FP22-pre-truncated on the host:
    # w_t[p, c*E + e] = trunc22(W[e, c*128 + p])
    w_t = nc.dram_tensor("wt", [P, HC * E], F32R, kind="ExternalInput")
    ident_in = nc.dram_tensor("ident", [P, P], F32R, kind="ExternalInput")
    o_probs = nc.dram_tensor("o_probs", [T, TOPK], F32, kind="ExternalOutput")
    o_idx = nc.dram_tensor("o_idx", [T, TOPK], U32, kind="ExternalOutput")
    o_m4 = nc.dram_tensor("o_m4", [T, 4], F32, kind="ExternalOutput")
    o_stats = nc.dram_tensor("o_stats", [P, 2 * E], F32, kind="ExternalOutput")

    with tile.TileContext(nc) as tc, ExitStack() as ctx:
        const = ctx.enter_context(tc.tile_pool(name="const", bufs=1))
        xpool = ctx.enter_context(tc.tile_pool(name="xpool", bufs=4))
        xtpool = ctx.enter_context(tc.tile_pool(name="xtpool", bufs=6))
        ps_t = ctx.enter_context(tc.tile_pool(name="ps_t", bufs=4, space="PSUM"))
        ps_l = ctx.enter_context(tc.tile_pool(name="ps_l", bufs=2, space="PSUM"))
        ps_lt = ctx.enter_context(tc.tile_pool(name="ps_lt", bufs=2, space="PSUM"))
        small = ctx.enter_context(tc.tile_pool(name="small", bufs=4))
        accp = ctx.enter_context(tc.tile_pool(name="accp", bufs=1))

        # Constants arrive via SWDGE so the HWDGE descriptor generator is
        # dedicated to the x stream (desc-gen, not bytes, is the DMA limit).
        ident = const.tile([P, P], F32R)
        nc.gpsimd.dma_start(out=ident[:, :], in_=ident_in[:, :])
        wt = const.tile([P, HC * E], F32R)
        nc.gpsimd.dma_start(out=wt[:, :], in_=w_t[:, :])

        # Token mapping: tile tt, partition p <-> token t = p*NT + tt.
        # Partition p's SUB group-tokens are then adjacent in DRAM, so one
        # DMA per group needs only 128 large (SUB*16KB) descriptors, and the
        # staged output DMAs are contiguous per partition too.
        def load_group(g):
            x_in = xpool.tile([P, SUB * H], F32R, tag="x", name=f"x_g{g}")
            if g == 0:
                # split so the first transposes start at half-load
                for s in range(SUB):
                    nc.sync.dma_start(
                        out=x_in[:, s * H:(s + 1) * H],
                        in_=bass.AP(
                            x, (g * SUB + s) * H, [[NT * H, P], [1, H]]
                        ),
                    )
            else:
                nc.sync.dma_start(
                    out=x_in[:, :],
                    in_=bass.AP(x, g * SUB * H, [[NT * H, P], [1, SUB * H]]),
                )
            return x_in

        count_acc = accp.tile([P, E], F32)
        prob_acc = accp.tile([P, E], F32)
        nc.vector.memset(count_acc[:, :], 0.0)
        nc.vector.memset(prob_acc[:, :], 0.0)

        st_probs = accp.tile([P, NT * TOPK], F32)
        st_idx = accp.tile([P, NT * TOPK], U32)
        st_m4 = accp.tile([P, NT * 4], F32)

        for g in range(NG):
            x_in = load_group(g)

            logits_ps = ps_l.tile([E, G], F32, tag="logits", name=f"lp{g}")
            for c in range(HC):
                tr_ps = ps_t.tile([P, G], F32, tag="tr", name=f"tr{g}_{c}")
                for s in range(SUB):
                    # f32r transpose: 1.5 cyc/col vs 2.0 for f32; the FP22
                    # truncation it applies is identical to what the f32r
                    # GEMM read does anyway.
                    nc.tensor.transpose(
                        tr_ps[:, s * P:(s + 1) * P].bitcast(F32R),
                        x_in[:, s * H + c * P:s * H + (c + 1) * P],
                        ident[:, :],
                    )
                xt_sb = xtpool.tile([P, G], F32R, tag="xt", name=f"xt{g}_{c}")
                if c % 2 == 0:
                    nc.scalar.copy(xt_sb[:, :], tr_ps[:, :])
                else:
                    nc.vector.tensor_copy(xt_sb[:, :], tr_ps[:, :])
                nc.tensor.matmul(
                    logits_ps[:, :],
                    wt[:, c * E:(c + 1) * E],
                    xt_sb[:, :],
                    start=(c == 0),
                    stop=(c == HC - 1),
                )

            logits_sb = small.tile([E, G], F32, tag="lsb", name=f"lsb{g}")
            nc.scalar.copy(logits_sb[:, :], logits_ps[:, :])

            for s in range(SUB):
                tt = g * SUB + s
                lt_ps = ps_lt.tile([P, E], F32, tag="lt_ps", name=f"ltp{tt}")
                nc.tensor.transpose(
                    lt_ps[:, :], logits_sb[:, s * P:(s + 1) * P],
                    ident[:E, :E].bitcast(F32),
                )
                lt = small.tile([P, E], F32, tag="lt", name=f"lt{tt}")
                nc.vector.tensor_copy(lt[:, :], lt_ps[:, :])

                maxs = small.tile([P, 8], F32, tag="maxs", name=f"mx{tt}")
                nc.vector.max(out=maxs[:, :], in_=lt[:, :])
                idx8 = small.tile([P, 8], U32, tag="idx8", name=f"ix{tt}")
                nc.vector.max_index(out=idx8[:, :], in_max=maxs[:, :], in_values=lt[:, :])

                nc.vector.tensor_copy(st_m4[:, tt * 4:(tt + 1) * 4], maxs[:, 0:4])
                nc.vector.tensor_copy(st_idx[:, tt * TOPK:(tt + 1) * TOPK], idx8[:, 0:TOPK])

                neg1 = small.tile([P, 1], F32, tag="neg1", name=f"n1{tt}")
                nc.vector.tensor_scalar_mul(neg1[:, :], maxs[:, 0:1], -1.0)

                exp_t = small.tile([P, E], F32, tag="expt", name=f"ex{tt}")
                denom = small.tile([P, 1], F32, tag="denom", name=f"dn{tt}")
                nc.scalar.activation(
                    exp_t[:, :], lt[:, :], mybir.ActivationFunctionType.Exp,
                    bias=neg1[:, :], scale=1.0, accum_out=denom[:, :],
                )
                rden = small.tile([P, 1], F32, tag="rden", name=f"rd{tt}")
                nc.vector.reciprocal(rden[:, :], denom[:, :])
                probs_t = small.tile([P, E], F32, tag="probst", name=f"pb{tt}")
                nc.vector.tensor_scalar_mul(probs_t[:, :], exp_t[:, :], rden[:, :])
                nc.vector.tensor_add(prob_acc[:, :], prob_acc[:, :], probs_t[:, :])

                mask = small.tile([P, E], F32, tag="mask", name=f"mk{tt}")
                nc.vector.tensor_scalar(
                    mask[:, :], lt[:, :], maxs[:, 1:2], None, op0=mybir.AluOpType.is_ge
                )
                nc.vector.tensor_add(count_acc[:, :], count_acc[:, :], mask[:, :])

                # renormalized top-2 probs: p1 = 1/(1+e2), p2 = e2/(1+e2)
                e2 = small.tile([P, 1], F32, tag="e2", name=f"e2{tt}")
                nc.scalar.activation(
                    e2[:, :], maxs[:, 1:2], mybir.ActivationFunctionType.Exp,
                    bias=neg1[:, :], scale=1.0,
                )
                dsum = small.tile([P, 1], F32, tag="dsum", name=f"ds{tt}")
                nc.vector.tensor_scalar_add(dsum[:, :], e2[:, :], 1.0)
                rsum = small.tile([P, 1], F32, tag="rsum", name=f"rs{tt}")
                nc.vector.reciprocal(rsum[:, :], dsum[:, :])
                nc.vector.tensor_copy(st_probs[:, tt * TOPK:tt * TOPK + 1], rsum[:, :])
                nc.vector.tensor_mul(
                    st_probs[:, tt * TOPK + 1:tt * TOPK + 2], e2[:, :], rsum[:, :]
                )

                # flush staged outputs every 4 tiles, via SWDGE (GpSimd is
                # otherwise idle) so SP/ACT stay on the critical path work
                if tt % 4 == 3:
                    lo = tt - 3
                    op = o_probs[:, :].rearrange("(p tt) k -> p (tt k)", p=P)
                    oi = o_idx[:, :].rearrange("(p tt) k -> p (tt k)", p=P)
                    om = o_m4[:, :].rearrange("(p tt) k -> p (tt k)", p=P)
                    nc.gpsimd.dma_start(
                        out=op[:, lo * TOPK:(tt + 1) * TOPK],
                        in_=st_probs[:, lo * TOPK:(tt + 1) * TOPK],
                    )
                    nc.gpsimd.dma_start(
                        out=oi[:, lo * TOPK:(tt + 1) * TOPK],
                        in_=st_idx[:, lo * TOPK:(tt + 1) * TOPK],
                    )
                    nc.gpsimd.dma_start(
                        out=om[:, lo * 4:(tt + 1) * 4],
                        in_=st_m4[:, lo * 4:(tt + 1) * 4],
                    )

        nc.gpsimd.dma_start(out=o_stats[:, 0:E], in_=count_acc[:, :])
        nc.gpsimd.dma_start(out=o_stats[:, E:2 * E], in_=prob_acc[:, :])

    nc.finalize()
    return nc


def get_nc():
    if "nc" not in _NC_CACHE:
        _NC_CACHE["nc"] = _build_nc()
    return _NC_CACHE["nc"]


def make_wt_host(Wf):
    """W [E,H] f32 -> [P, HC*E] with wt[p, c*E+e] = trunc22(W[e, c*128+p])."""
    Wt = np.ascontiguousarray(Wf, dtype=np.float32).copy()
    Wt.view(np.uint32)[...] &= np.uint32(0xFFFFFC00)  # FP22 truncation
    return np.ascontiguousarray(
        Wt.reshape(E, HC, P).transpose(2, 1, 0).reshape(P, HC * E)
    )


def _host_postprocess(xt, Wf, probs, idx, m4, counts, psums):
    """Fix tokens whose FP22 top-3 margins are within noise of a tie."""
    thr = 2e-3
    risk = ((m4[:, 0] - m4[:, 1]) < thr) | ((m4[:, 1] - m4[:, 2]) < thr)
    ridx = np.nonzero(risk)[0]
    if ridx.size:
        lg = xt[ridx].astype(np.float64) @ Wf.astype(np.float64).T  # [n, E]
        order = np.argsort(-lg, axis=1, kind="stable")[:, :TOPK].astype(np.int32)
        lsel = np.take_along_axis(lg, order, 1)
        ex = np.exp(lsel - lsel[:, :1])
        pn = ex / ex.sum(1, keepdims=True)
        np.add.at(counts, idx[ridx].reshape(-1), -1.0)
        np.add.at(counts, order.reshape(-1), 1.0)
        idx[ridx] = order
        probs[ridx] = pn.astype(np.float32)
    aux = E * np.sum((counts / B) * (psums / (B * S)))
    return probs, idx, np.float32(aux)


def kernel(x, W):
    from concourse.bass_utils import run_bass_kernel_spmd

    xt = np.ascontiguousarray(np.asarray(x, dtype=np.float32).reshape(B * S, H))
    Wf = np.ascontiguousarray(np.asarray(W, dtype=np.float32))

    nc = get_nc()
    wt_host = make_wt_host(Wf)
    ident_host = np.eye(P, dtype=np.float32)
    in_maps = [
        {"x": xt[i * T:(i + 1) * T], "wt": wt_host, "ident": ident_host}
        for i in range(NCORES)
    ]
    res = run_bass_kernel_spmd(nc, in_maps, core_ids=list(range(NCORES)))

    probs = np.concatenate([r["o_probs"] for r in res.results], axis=0)
    idx = np.concatenate([r["o_idx"] for r in res.results], axis=0).astype(np.int32)
    m4 = np.concatenate([r["o_m4"] for r in res.results], axis=0)
    stats = np.stack([r["o_stats"] for r in res.results])  # [8, P, 2E]
    counts = stats[:, :, :E].sum(axis=(0, 1)).astype(np.float64)
    psums = stats[:, :, E:].sum(axis=(0, 1)).astype(np.float64)

    probs, idx, aux = _host_postprocess(xt, Wf, probs, idx, m4, counts, psums)
    return (
        probs.reshape(B, S, TOPK),
        idx.reshape(B, S, TOPK),
        aux,
    )


# revision 40
# speedup vs baseline: 1.0675x; 1.0466x over previous
"""DiT MoE router kernel for Trainium2 (8 NeuronCores, SPMD).

Computes, for x [B,S,H] and gate weight W [E,H]:
  logits = x @ W.T              (gate GEMM, E=16)
  probs  = softmax(logits)
  top2 probs (renormalized) + expert indices + load-balancing aux loss.

Sharding: tokens (B*S) split into 8 contiguous slabs of 2048, one per core;
W replicated. Each core returns its top-2 probs/indices plus per-partition
count/prob-sum accumulators; the host reduces the [E]-sized stats and forms
the aux-loss scalar (the cross-core reduction is 8*2*16 floats, so an
on-device collective would only add latency).

Device GEMM runs in float32r (FP22 multiplies, fp32 accumulate). Tokens whose
top-3 logit margins are below a safety threshold (~1e-3, vs FP22 logit error
~2e-4) are recomputed exactly on the host in float64 — a few dozen tokens out
of 16K — so returned indices match a full-fp32 reference everywhere the
reference itself is numerically well-defined.
"""

import numpy as np

import concourse.bass as bass
import concourse.mybir as mybir
import concourse.tile as tile
from concourse import bacc

NCORES = 8
B, S, H, E, TOPK = 4, 4096, 4096, 16, 2
T = (B * S) // NCORES  # tokens per core (2048)
P = 128                # partitions
NT = T // P            # token tiles per core (16)
HC = H // P            # contraction chunks (32)
G = 256                # tokens per GEMM group (matmul free dim)
SUB = G // P           # token tiles per group (2)
NG = T // G            # groups per core (8)

F32 = mybir.dt.float32
F32R = mybir.dt.float32r
U32 = mybir.dt.uint32

_NC_CACHE = {}


def _build_nc():
    from contextlib import ExitStack

    # Bacc (not plain Bass): its compile() pass splits multi-wait sync
    # conditions, which walrus requires (max 1 wait per instruction).
    nc = bacc.Bacc(None)
    # x declared float32r end-to-end: the f32r transposes consume it, and the
    # BIR verifier requires f32r consumers to see f32r-typed producers.
    # Bytes are plain f32; the PE read path truncates to FP22.
    x = nc.dram_tensor("x", [T, H], F32R, kind="ExternalInput")
    # W^T pre-interleaved and FP22-pre-truncated on the host:
    # w_t[p, c*E + e] = trunc22(W[e, c*128 + p])
    w_t = nc.dram_tensor("wt", [P, HC * E], F32R, kind="ExternalInput")
    ident_in = nc.dram_tensor("ident", [P, P], F32R, kind="ExternalInput")
    o_probs = nc.dram_tensor("o_probs", [T, TOPK], F32, kind="ExternalOutput")
    o_idx = nc.dram_tensor("o_idx", [T, TOPK], U32, kind="ExternalOutput")
    o_m4 = nc.dram_tensor("o_m4", [T, 4], F32, kind="ExternalOutput")
    o_stats = nc.dram_tensor("o_stats", [P, 2 * E], F32, kind="ExternalOutput")

    with tile.TileContext(nc) as tc, ExitStack() as ctx:
        const = ctx.enter_context(tc.tile_pool(name="const", bufs=1))
        xpool = ctx.enter_context(tc.tile_pool(name="xpool", bufs=4))
        xtpool = ctx.enter_context(tc.tile_pool(name="xtpool", bufs=6))
        ps_t = ctx.enter_context(tc.tile_pool(name="ps_t", bufs=4, space="PSUM"))
        ps_l = ctx.enter_context(tc.tile_pool(name="ps_l", bufs=2, space="PSUM"))
        ps_lt = ctx.enter_context(tc.tile_pool(name="ps_lt", bufs=2, space="PSUM"))
        small = ctx.enter_context(tc.tile_pool(name="small", bufs=4))
        accp = ctx.enter_context(tc.tile_pool(name="accp", bufs=1))

        # Small constant loads go first on HWDGE (~2us), ahead of the x
        # stream; SWDGE would gate the pipeline start on slow GpSimd
        # descriptor writing instead.
        ident = const.tile([P, P], F32R)
        nc.sync.dma_start(out=ident[:, :], in_=ident_in[:, :])
        wt = const.tile([P, HC * E], F32R)
        nc.sync.dma_start(out=wt[:, :], in_=w_t[:, :])

        # Token mapping: tile tt, partition p <-> token t = p*NT + tt.
        # Partition p's SUB group-tokens are then adjacent in DRAM, so one
        # DMA per group needs only 128 large (SUB*16KB) descriptors, and the
        # staged output DMAs are contiguous per partition too.
        def load_group(g):
            x_in = xpool.tile([P, SUB * H], F32R, tag="x", name=f"x_g{g}")
            if g == 0:
                # split so the first transposes start at half-load
                for s in range(SUB):
                    nc.sync.dma_start(
                        out=x_in[:, s * H:(s + 1) * H],
                        in_=bass.AP(
                            x, (g * SUB + s) * H, [[NT * H, P], [1, H]]
                        ),
                    )
            else:
                nc.sync.dma_start(
                    out=x_in[:, :],
                    in_=bass.AP(x, g * SUB * H, [[NT * H, P], [1, SUB * H]]),
                )
            return x_in

        count_acc = accp.tile([P, E], F32)
        prob_acc = accp.tile([P, E], F32)
        nc.vector.memset(count_acc[:, :], 0.0)
        nc.vector.memset(prob_acc[:, :], 0.0)

        st_probs = accp.tile([P, NT * TOPK], F32)
        st_idx = accp.tile([P, NT * TOPK], U32)
        st_m4 = accp.tile([P, NT * 4], F32)

        for g in range(NG):
            x_in = load_group(g)

            logits_ps = ps_l.tile([E, G], F32, tag="logits", name=f"lp{g}")
            for c in range(HC):
                tr_ps = ps_t.tile([P, G], F32, tag="tr", name=f"tr{g}_{c}")
                for s in range(SUB):
                    # f32r transpose: 1.5 cyc/col vs 2.0 for f32; the FP22
                    # truncation it applies is identical to what the f32r
                    # GEMM read does anyway.
                    nc.tensor.transpose(
                        tr_ps[:, s * P:(s + 1) * P].bitcast(F32R),
                        x_in[:, s * H + c * P:s * H + (c + 1) * P],
                        ident[:, :],
                    )
                xt_sb = xtpool.tile([P, G], F32R, tag="xt", name=f"xt{g}_{c}")
                if c % 2 == 0:
                    nc.scalar.copy(xt_sb[:, :], tr_ps[:, :])
                else:
                    nc.vector.tensor_copy(xt_sb[:, :], tr_ps[:, :])
                nc.tensor.matmul(
                    logits_ps[:, :],
                    wt[:, c * E:(c + 1) * E],
                    xt_sb[:, :],
                    start=(c == 0),
                    stop=(c == HC - 1),
                )

            logits_sb = small.tile([E, G], F32, tag="lsb", name=f"lsb{g}")
            nc.scalar.copy(logits_sb[:, :], logits_ps[:, :])

            for s in range(SUB):
                tt = g * SUB + s
                lt_ps = ps_lt.tile([P, E], F32, tag="lt_ps", name=f"ltp{tt}")
                nc.tensor.transpose(
                    lt_ps[:, :], logits_sb[:, s * P:(s + 1) * P],
                    ident[:E, :E].bitcast(F32),
                )
                lt = small.tile([P, E], F32, tag="lt", name=f"lt{tt}")
                nc.vector.tensor_copy(lt[:, :], lt_ps[:, :])

                maxs = small.tile([P, 8], F32, tag="maxs", name=f"mx{tt}")
                nc.vector.max(out=maxs[:, :], in_=lt[:, :])
                idx8 = small.tile([P, 8], U32, tag="idx8", name=f"ix{tt}")
                nc.vector.max_index(out=idx8[:, :], in_max=maxs[:, :], in_values=lt[:, :])

                nc.vector.tensor_copy(st_m4[:, tt * 4:(tt + 1) * 4], maxs[:, 0:4])
                nc.vector.tensor_copy(st_idx[:, tt * TOPK:(tt + 1) * TOPK], idx8[:, 0:TOPK])

                neg1 = small.tile([P, 1], F32, tag="neg1", name=f"n1{tt}")
                nc.vector.tensor_scalar_mul(neg1[:, :], maxs[:, 0:1], -1.0)

                exp_t = small.tile([P, E], F32, tag="expt", name=f"ex{tt}")
                denom = small.tile([P, 1], F32, tag="denom", name=f"dn{tt}")
                nc.scalar.activation(
                    exp_t[:, :], lt[:, :], mybir.ActivationFunctionType.Exp,
                    bias=neg1[:, :], scale=1.0, accum_out=denom[:, :],
                )
                rden = small.tile([P, 1], F32, tag="rden", name=f"rd{tt}")
                nc.vector.reciprocal(rden[:, :], denom[:, :])
                probs_t = small.tile([P, E], F32, tag="probst", name=f"pb{tt}")
                nc.vector.tensor_scalar_mul(probs_t[:, :], exp_t[:, :], rden[:, :])
                nc.vector.tensor_add(prob_acc[:, :], prob_acc[:, :], probs_t[:, :])

                mask = small.tile([P, E], F32, tag="mask", name=f"mk{tt}")
                nc.vector.tensor_scalar(
                    mask[:, :], lt[:, :], maxs[:, 1:2], None, op0=mybir.AluOpType.is_ge
                )
                nc.vector.tensor_add(count_acc[:, :], count_acc[:, :], mask[:, :])

                # renormalized top-2 probs: p1 = 1/(1+e2), p2 = e2/(1+e2)
                e2 = small.tile([P, 1], F32, tag="e2", name=f"e2{tt}")
                nc.scalar.activation(
                    e2[:, :], maxs[:, 1:2], mybir.ActivationFunctionType.Exp,
                    bias=neg1[:, :], scale=1.0,
                )
                dsum = small.tile([P, 1], F32, tag="dsum", name=f"ds{tt}")
                nc.vector.tensor_scalar_add(dsum[:, :], e2[:, :], 1.0)
                rsum = small.tile([P, 1], F32, tag="rsum", name=f"rs{tt}")
                nc.vector.reciprocal(rsum[:, :], dsum[:, :])
                nc.vector.tensor_copy(st_probs[:, tt * TOPK:tt * TOPK + 1], rsum[:, :])
                nc.vector.tensor_mul(
                    st_probs[:, tt * TOPK + 1:tt * TOPK + 2], e2[:, :], rsum[:, :]
                )

                # flush staged outputs every 4 tiles, via SWDGE (GpSimd is
                # otherwise idle) so SP/ACT stay on the critical path work
                if tt % 4 == 3:
                    lo = tt - 3
                    op = o_probs[:, :].rearrange("(p tt) k -> p (tt k)", p=P)
                    oi = o_idx[:, :].rearrange("(p tt) k -> p (tt k)", p=P)
                    om = o_m4[:, :].rearrange("(p tt) k -> p (tt k)", p=P)
                    nc.gpsimd.dma_start(
                        out=op[:, lo * TOPK:(tt + 1) * TOPK],
                        in_=st_probs[:, lo * TOPK:(tt + 1) * TOPK],
                    )
                    nc.gpsimd.dma_start(
                        out=oi[:, lo * TOPK:(tt + 1) * TOPK],
                        in_=st_idx[:, lo * TOPK:(tt + 1) * TOPK],
                    )
                    nc.gpsimd.dma_start(
                        out=om[:, lo * 4:(tt + 1) * 4],
                        in_=st_m4[:, lo * 4:(tt + 1) * 4],
                    )

        nc.gpsimd.dma_start(out=o_stats[:, 0:E], in_=count_acc[:, :])
        nc.gpsimd.dma_start(out=o_stats[:, E:2 * E], in_=prob_acc[:, :])

    nc.finalize()
    return nc


def get_nc():
    if "nc" not in _NC_CACHE:
        _NC_CACHE["nc"] = _build_nc()
    return _NC_CACHE["nc"]


def make_wt_host(Wf):
    """W [E,H] f32 -> [P, HC*E] with wt[p, c*E+e] = trunc22(W[e, c*128+p])."""
    Wt = np.ascontiguousarray(Wf, dtype=np.float32).copy()
    Wt.view(np.uint32)[...] &= np.uint32(0xFFFFFC00)  # FP22 truncation
    return np.ascontiguousarray(
        Wt.reshape(E, HC, P).transpose(2, 1, 0).reshape(P, HC * E)
    )


def _host_postprocess(xt, Wf, probs, idx, m4, counts, psums):
    """Fix tokens whose FP22 top-3 margins are within noise of a tie."""
    thr = 2e-3
    risk = ((m4[:, 0] - m4[:, 1]) < thr) | ((m4[:, 1] - m4[:, 2]) < thr)
    ridx = np.nonzero(risk)[0]
    if ridx.size:
        lg = xt[ridx].astype(np.float64) @ Wf.astype(np.float64).T  # [n, E]
        order = np.argsort(-lg, axis=1, kind="stable")[:, :TOPK].astype(np.int32)
        lsel = np.take_along_axis(lg, order, 1)
        ex = np.exp(lsel - lsel[:, :1])
        pn = ex / ex.sum(1, keepdims=True)
        np.add.at(counts, idx[ridx].reshape(-1), -1.0)
        np.add.at(counts, order.reshape(-1), 1.0)
        idx[ridx] = order
        probs[ridx] = pn.astype(np.float32)
    aux = E * np.sum((counts / B) * (psums / (B * S)))
    return probs, idx, np.float32(aux)


def kernel(x, W):
    from concourse.bass_utils import run_bass_kernel_spmd

    xt = np.ascontiguousarray(np.asarray(x, dtype=np.float32).reshape(B * S, H))
    Wf = np.ascontiguousarray(np.asarray(W, dtype=np.float32))

    nc = get_nc()
    wt_host = make_wt_host(Wf)
    ident_host = np.eye(P, dtype=np.float32)
    in_maps = [
        {"x": xt[i * T:(i + 1) * T], "wt": wt_host, "ident": ident_host}
        for i in range(NCORES)
    ]
    res = run_bass_kernel_spmd(nc, in_maps, core_ids=list(range(NCORES)))

    probs = np.concatenate([r["o_probs"] for r in res.results], axis=0)
    idx = np.concatenate([r["o_idx"] for r in res.results], axis=0).astype(np.int32)
    m4 = np.concatenate([r["o_m4"] for r in res.results], axis=0)
    stats = np.stack([r["o_stats"] for r in res.results])  # [8, P, 2E]
    counts = stats[:, :, :E].sum(axis=(0, 1)).astype(np.float64)
    psums = stats[:, :, E:].sum(axis=(0, 1)).astype(np.float64)

    probs, idx, aux = _host_postprocess(xt, Wf, probs, idx, m4, counts, psums)
    return (
        probs.reshape(B, S, TOPK),
        idx.reshape(B, S, TOPK),
        aux,
    )


# revision 50
# speedup vs baseline: 1.2336x; 1.1555x over previous
"""DiT MoE router kernel for Trainium2 (8 NeuronCores, SPMD).

Computes, for x [B,S,H] and gate weight W [E,H]:
  logits = x @ W.T              (gate GEMM, E=16)
  probs  = softmax(logits)
  top2 probs (renormalized) + expert indices + load-balancing aux loss.

Sharding: tokens (B*S) split into 8 contiguous slabs of 2048, one per core;
W replicated. Each core returns its top-2 probs/indices plus per-partition
count/prob-sum accumulators; the host reduces the [E]-sized stats and forms
the aux-loss scalar (the cross-core reduction is 8*2*16 floats, so an
on-device collective would only add latency).

Device GEMM runs in float32r (FP22 multiplies, fp32 accumulate). Tokens whose
top-3 logit margins are below a safety threshold (~1e-3, vs FP22 logit error
~2e-4) are recomputed exactly on the host in float64 — a few dozen tokens out
of 16K — so returned indices match a full-fp32 reference everywhere the
reference itself is numerically well-defined.
"""

import numpy as np

import concourse.bass as bass
import concourse.mybir as mybir
import concourse.tile as tile
from concourse import bacc

NCORES = 8
B, S, H, E, TOPK = 4, 4096, 4096, 16, 2
T = (B * S) // NCORES  # tokens per core (2048)
P = 128                # partitions
NT = T // P            # token tiles per core (16)
HC = H // P            # contraction chunks (32)
G = 256                # tokens per GEMM group (matmul free dim)
SUB = G // P           # token tiles per group (2)
NG = T // G            # groups per core (8)

F32 = mybir.dt.float32
F32R = mybir.dt.float32r
U32 = mybir.dt.uint32

_NC_CACHE = {}


def _build_nc():
    from contextlib import ExitStack

    # Bacc (not plain Bass): its compile() pass splits multi-wait sync
    # conditions, which walrus requires (max 1 wait per instruction).
    nc = bacc.Bacc(None)
    # x declared float32r end-to-end: the f32r transposes consume it, and the
    # BIR verifier requires f32r consumers to see f32r-typed producers.
    # Bytes are plain f32; the PE read path truncates to FP22.
    x = nc.dram_tensor("x", [T, H], F32R, kind="ExternalInput")
    # W^T pre-interleaved and FP22-pre-truncated on the host:
    # w_t[p, c*E + e] = trunc22(W[e, c*128 + p])
    w_t = nc.dram_tensor("wt", [P, HC * E], F32R, kind="ExternalInput")
    ident_in = nc.dram_tensor("ident", [P, P], F32R, kind="ExternalInput")
    o_probs = nc.dram_tensor("o_probs", [T, TOPK], F32, kind="ExternalOutput")
    o_idx = nc.dram_tensor("o_idx", [T, TOPK], U32, kind="ExternalOutput")
    o_m4 = nc.dram_tensor("o_m4", [T, 4], F32, kind="ExternalOutput")
    o_stats = nc.dram_tensor("o_stats", [P, 2 * E], F32, kind="ExternalOutput")

    with tile.TileContext(nc) as tc, ExitStack() as ctx:
        const = ctx.enter_context(tc.tile_pool(name="const", bufs=1))
        xpool = ctx.enter_context(tc.tile_pool(name="xpool", bufs=4))
        xtpool = ctx.enter_context(tc.tile_pool(name="xtpool", bufs=6))
        ps_t = ctx.enter_context(tc.tile_pool(name="ps_t", bufs=6, space="PSUM"))
        ps_l = ctx.enter_context(tc.tile_pool(name="ps_l", bufs=2, space="PSUM"))
        small = ctx.enter_context(tc.tile_pool(name="small", bufs=6))
        accp = ctx.enter_context(tc.tile_pool(name="accp", bufs=1))

        ident = const.tile([P, P], F32R)
        wt = const.tile([P, HC * E], F32R)

        # Token mapping: tile tt, partition p <-> token t = p*NT + tt.
        # Partition p's SUB group-tokens are then adjacent in DRAM, so one
        # DMA per group needs only 128 large (SUB*16KB) descriptors, and the
        # staged output DMAs are contiguous per partition too.
        def load_group(g):
            x_in = xpool.tile([P, SUB * H], F32R, tag="x", name=f"x_g{g}")
            if g == 0:
                # 4 half-H pieces so the first transposes start early; small
                # constant loads slot in right after piece 0 (HWDGE -- SWDGE
                # would gate the start on slow GpSimd descriptor writes)
                H2 = H // 2
                first = True
                nc.sync.dma_start(out=ident[:, :], in_=ident_in[:, :])
                for half in range(2):
                    for s in range(SUB):
                        nc.sync.dma_start(
                            out=x_in[:, s * H + half * H2:s * H + half * H2 + H2],
                            in_=bass.AP(
                                x, s * H + half * H2, [[NT * H, P], [1, H2]]
                            ),
                        )
                        if first:
                            first = False
                            nc.sync.dma_start(out=wt[:, :], in_=w_t[:, :])
            else:
                nc.sync.dma_start(
                    out=x_in[:, :],
                    in_=bass.AP(x, g * SUB * H, [[NT * H, P], [1, SUB * H]]),
                )
            return x_in

        count_acc = accp.tile([P, E], F32)
        prob_acc = accp.tile([P, E], F32)
        nc.vector.memset(count_acc[:, :], 0.0)
        nc.vector.memset(prob_acc[:, :], 0.0)

        st_probs = accp.tile([P, NT * TOPK], F32)
        st_idx = accp.tile([P, NT * TOPK], U32)
        st_m4 = accp.tile([P, NT * 4], F32)

        def postprocess(g, logits_sb):
            for s in range(SUB):
                tt = g * SUB + s
                lt_ps = ps_t.tile([P, E], F32, tag="tr", name=f"ltp{tt}")
                nc.tensor.transpose(
                    lt_ps[:, :], logits_sb[:, s * P:(s + 1) * P],
                    ident[:E, :E].bitcast(F32),
                )
                lt = small.tile([P, E], F32, tag="lt", name=f"lt{tt}")
                nc.vector.tensor_copy(lt[:, :], lt_ps[:, :])

                maxs = small.tile([P, 8], F32, tag="maxs", name=f"mx{tt}")
                nc.vector.max(out=maxs[:, :], in_=lt[:, :])
                idx8 = small.tile([P, 8], U32, tag="idx8", name=f"ix{tt}")
                nc.vector.max_index(out=idx8[:, :], in_max=maxs[:, :], in_values=lt[:, :])

                nc.vector.tensor_copy(st_m4[:, tt * 4:(tt + 1) * 4], maxs[:, 0:4])
                nc.vector.tensor_copy(st_idx[:, tt * TOPK:(tt + 1) * TOPK], idx8[:, 0:TOPK])

                neg1 = small.tile([P, 1], F32, tag="neg1", name=f"n1{tt}")
                nc.vector.tensor_scalar_mul(neg1[:, :], maxs[:, 0:1], -1.0)

                exp_t = small.tile([P, E], F32, tag="expt", name=f"ex{tt}")
                denom = small.tile([P, 1], F32, tag="denom", name=f"dn{tt}")
                nc.scalar.activation(
                    exp_t[:, :], lt[:, :], mybir.ActivationFunctionType.Exp,
                    bias=neg1[:, :], scale=1.0, accum_out=denom[:, :],
                )
                rden = small.tile([P, 1], F32, tag="rden", name=f"rd{tt}")
                nc.vector.reciprocal(rden[:, :], denom[:, :])
                probs_t = small.tile([P, E], F32, tag="probst", name=f"pb{tt}")
                nc.vector.tensor_scalar_mul(probs_t[:, :], exp_t[:, :], rden[:, :])
                nc.vector.tensor_add(prob_acc[:, :], prob_acc[:, :], probs_t[:, :])

                mask = small.tile([P, E], F32, tag="mask", name=f"mk{tt}")
                nc.vector.tensor_scalar(
                    mask[:, :], lt[:, :], maxs[:, 1:2], None, op0=mybir.AluOpType.is_ge
                )
                nc.vector.tensor_add(count_acc[:, :], count_acc[:, :], mask[:, :])

                # renormalized top-2 probs: p1 = 1/(1+e2), p2 = e2/(1+e2)
                e2 = small.tile([P, 1], F32, tag="e2", name=f"e2{tt}")
                nc.scalar.activation(
                    e2[:, :], maxs[:, 1:2], mybir.ActivationFunctionType.Exp,
                    bias=neg1[:, :], scale=1.0,
                )
                dsum = small.tile([P, 1], F32, tag="dsum", name=f"ds{tt}")
                nc.vector.tensor_scalar_add(dsum[:, :], e2[:, :], 1.0)
                rsum = small.tile([P, 1], F32, tag="rsum", name=f"rs{tt}")
                nc.vector.reciprocal(rsum[:, :], dsum[:, :])
                nc.vector.tensor_copy(st_probs[:, tt * TOPK:tt * TOPK + 1], rsum[:, :])
                nc.vector.tensor_mul(
                    st_probs[:, tt * TOPK + 1:tt * TOPK + 2], e2[:, :], rsum[:, :]
                )

                # flush staged outputs every 4 tiles, via SWDGE (GpSimd is
                # otherwise idle) so SP/ACT stay on the critical path work
                if tt % 4 == 3:
                    lo = tt - 3
                    op = o_probs[:, :].rearrange("(p tt) k -> p (tt k)", p=P)
                    oi = o_idx[:, :].rearrange("(p tt) k -> p (tt k)", p=P)
                    om = o_m4[:, :].rearrange("(p tt) k -> p (tt k)", p=P)
                    nc.gpsimd.dma_start(
                        out=op[:, lo * TOPK:(tt + 1) * TOPK],
                        in_=st_probs[:, lo * TOPK:(tt + 1) * TOPK],
                    )
                    nc.gpsimd.dma_start(
                        out=oi[:, lo * TOPK:(tt + 1) * TOPK],
                        in_=st_idx[:, lo * TOPK:(tt + 1) * TOPK],
                    )
                    nc.gpsimd.dma_start(
                        out=om[:, lo * 4:(tt + 1) * 4],
                        in_=st_m4[:, lo * 4:(tt + 1) * 4],
                    )


        pending = []

        for g in range(NG):
            x_in = load_group(g)

            logits_ps = ps_l.tile([E, G], F32, tag="logits", name=f"lp{g}")
            for c in range(HC):
                if c == 4 and pending:
                    postprocess(*pending.pop(0))
                tr_ps = ps_t.tile([P, G], F32, tag="tr", name=f"tr{g}_{c}")
                for s in range(SUB):
                    # f32r transpose: 1.5 cyc/col vs 2.0 for f32; the FP22
                    # truncation it applies is identical to what the f32r
                    # GEMM read does anyway.
                    nc.tensor.transpose(
                        tr_ps[:, s * P:(s + 1) * P].bitcast(F32R),
                        x_in[:, s * H + c * P:s * H + (c + 1) * P],
                        ident[:, :],
                    )
                xt_sb = xtpool.tile([P, G], F32R, tag="xt", name=f"xt{g}_{c}")
                if c % 2 == 0:
                    nc.scalar.copy(xt_sb[:, :], tr_ps[:, :])
                else:
                    nc.vector.tensor_copy(xt_sb[:, :], tr_ps[:, :])
                nc.tensor.matmul(
                    logits_ps[:, :],
                    wt[:, c * E:(c + 1) * E],
                    xt_sb[:, :],
                    start=(c == 0),
                    stop=(c == HC - 1),
                )

            logits_sb = small.tile([E, G], F32, tag="lsb", name=f"lsb{g}")
            nc.scalar.copy(logits_sb[:, :], logits_ps[:, :])
            pending.append((g, logits_sb))

        for item in pending:
            postprocess(*item)

        nc.gpsimd.dma_start(out=o_stats[:, 0:E], in_=count_acc[:, :])
        nc.gpsimd.dma_start(out=o_stats[:, E:2 * E], in_=prob_acc[:, :])

    nc.finalize()
    return nc


def get_nc():
    if "nc" not in _NC_CACHE:
        _NC_CACHE["nc"] = _build_nc()
    return _NC_CACHE["nc"]


def make_wt_host(Wf):
    """W [E,H] f32 -> [P, HC*E] with wt[p, c*E+e] = trunc22(W[e, c*128+p])."""
    Wt = np.ascontiguousarray(Wf, dtype=np.float32).copy()
    Wt.view(np.uint32)[...] &= np.uint32(0xFFFFFC00)  # FP22 truncation
    return np.ascontiguousarray(
        Wt.reshape(E, HC, P).transpose(2, 1, 0).reshape(P, HC * E)
    )


def _host_postprocess(xt, Wf, probs, idx, m4, counts, psums):
    """Fix tokens whose FP22 top-3 margins are within noise of a tie."""
    thr = 2e-3
    risk = ((m4[:, 0] - m4[:, 1]) < thr) | ((m4[:, 1] - m4[:, 2]) < thr)
    ridx = np.nonzero(risk)[0]
    if ridx.size:
        lg = xt[ridx].astype(np.float64) @ Wf.astype(np.float64).T  # [n, E]
        order = np.argsort(-lg, axis=1, kind="stable")[:, :TOPK].astype(np.int32)
        lsel = np.take_along_axis(lg, order, 1)
        ex = np.exp(lsel - lsel[:, :1])
        pn = ex / ex.sum(1, keepdims=True)
        np.add.at(counts, idx[ridx].reshape(-1), -1.0)
        np.add.at(counts, order.reshape(-1), 1.0)
        idx[ridx] = order
        probs[ridx] = pn.astype(np.float32)
    aux = E * np.sum((counts / B) * (psums / (B * S)))
    return probs, idx, np.float32(aux)


def kernel(x, W):
    from concourse.bass_utils import run_bass_kernel_spmd

    xt = np.ascontiguousarray(np.asarray(x, dtype=np.float32).reshape(B * S, H))
    Wf = np.ascontiguousarray(np.asarray(W, dtype=np.float32))

    nc = get_nc()
    wt_host = make_wt_host(Wf)
    ident_host = np.eye(P, dtype=np.float32)
    in_maps = [
        {"x": xt[i * T:(i + 1) * T], "wt": wt_host, "ident": ident_host}
        for i in range(NCORES)
    ]
    res = run_bass_kernel_spmd(nc, in_maps, core_ids=list(range(NCORES)))

    probs = np.concatenate([r["o_probs"] for r in res.results], axis=0)
    idx = np.concatenate([r["o_idx"] for r in res.results], axis=0).astype(np.int32)
    m4 = np.concatenate([r["o_m4"] for r in res.results], axis=0)
    stats = np.stack([r["o_stats"] for r in res.results])  # [8, P, 2E]
    counts = stats[:, :, :E].sum(axis=(0, 1)).astype(np.float64)
    psums = stats[:, :, E:].sum(axis=(0, 1)).astype(np.float64)

    probs, idx, aux = _host_postprocess(xt, Wf, probs, idx, m4, counts, psums)
    return (
        probs.reshape(B, S, TOPK),
        idx.reshape(B, S, TOPK),
        aux,
    )


# revision 51
# speedup vs baseline: 1.2382x; 1.0037x over previous
"""DiT MoE router kernel for Trainium2 (8 NeuronCores, SPMD).

Computes, for x [B,S,H] and gate weight W [E,H]:
  logits = x @ W.T              (gate GEMM, E=16)
  probs  = softmax(logits)
  top2 probs (renormalized) + expert indices + load-balancing aux loss.

Sharding: tokens (B*S) split into 8 contiguous slabs of 2048, one per core;
W replicated. Each core returns its top-2 probs/indices plus per-partition
count/prob-sum accumulators; the host reduces the [E]-sized stats and forms
the aux-loss scalar (the cross-core reduction is 8*2*16 floats, so an
on-device collective would only add latency).

Device GEMM runs in float32r (FP22 multiplies, fp32 accumulate). Tokens whose
top-3 logit margins are below a safety threshold (~1e-3, vs FP22 logit error
~2e-4) are recomputed exactly on the host in float64 — a few dozen tokens out
of 16K — so returned indices match a full-fp32 reference everywhere the
reference itself is numerically well-defined.
"""

import numpy as np

import concourse.bass as bass
import concourse.mybir as mybir
import concourse.tile as tile
from concourse import bacc

NCORES = 8
B, S, H, E, TOPK = 4, 4096, 4096, 16, 2
T = (B * S) // NCORES  # tokens per core (2048)
P = 128                # partitions
NT = T // P            # token tiles per core (16)
HC = H // P            # contraction chunks (32)
G = 256                # tokens per GEMM group (matmul free dim)
SUB = G // P           # token tiles per group (2)
NG = T // G            # groups per core (8)

F32 = mybir.dt.float32
F32R = mybir.dt.float32r
U32 = mybir.dt.uint32

_NC_CACHE = {}


def _build_nc():
    from contextlib import ExitStack

    # Bacc (not plain Bass): its compile() pass splits multi-wait sync
    # conditions, which walrus requires (max 1 wait per instruction).
    nc = bacc.Bacc(None)
    # x declared float32r end-to-end: the f32r transposes consume it, and the
    # BIR verifier requires f32r consumers to see f32r-typed producers.
    # Bytes are plain f32; the PE read path truncates to FP22.
    x = nc.dram_tensor("x", [T, H], F32R, kind="ExternalInput")
    # W^T pre-interleaved and FP22-pre-truncated on the host:
    # w_t[p, c*E + e] = trunc22(W[e, c*128 + p])
    w_t = nc.dram_tensor("wt", [P, HC * E], F32R, kind="ExternalInput")
    ident_in = nc.dram_tensor("ident", [P, P], F32R, kind="ExternalInput")
    o_probs = nc.dram_tensor("o_probs", [T, TOPK], F32, kind="ExternalOutput")
    o_idx = nc.dram_tensor("o_idx", [T, TOPK], U32, kind="ExternalOutput")
    o_m4 = nc.dram_tensor("o_m4", [T, 4], F32, kind="ExternalOutput")
    o_stats = nc.dram_tensor("o_stats", [P, 2 * E], F32, kind="ExternalOutput")

    with tile.TileContext(nc) as tc, ExitStack() as ctx:
        const = ctx.enter_context(tc.tile_pool(name="const", bufs=1))
        xpool = ctx.enter_context(tc.tile_pool(name="xpool", bufs=4))
        xtpool = ctx.enter_context(tc.tile_pool(name="xtpool", bufs=6))
        ps_t = ctx.enter_context(tc.tile_pool(name="ps_t", bufs=8, space="PSUM"))
        small = ctx.enter_context(tc.tile_pool(name="small", bufs=6))
        accp = ctx.enter_context(tc.tile_pool(name="accp", bufs=1))

        ident = const.tile([P, P], F32R)
        wt = const.tile([P, HC * E], F32R)

        # Token mapping: tile tt, partition p <-> token t = p*NT + tt.
        # Partition p's SUB group-tokens are then adjacent in DRAM, so one
        # DMA per group needs only 128 large (SUB*16KB) descriptors, and the
        # staged output DMAs are contiguous per partition too.
        def load_group(g):
            x_in = xpool.tile([P, SUB * H], F32R, tag="x", name=f"x_g{g}")
            if g == 0:
                # 4 half-H pieces so the first transposes start early; small
                # constant loads slot in right after piece 0 (HWDGE -- SWDGE
                # would gate the start on slow GpSimd descriptor writes)
                H2 = H // 2
                nc.sync.dma_start(out=ident[:, :], in_=ident_in[:, :])
                for half in range(2):
                    for s in range(SUB):
                        nc.sync.dma_start(
                            out=x_in[:, s * H + half * H2:s * H + half * H2 + H2],
                            in_=bass.AP(
                                x, s * H + half * H2, [[NT * H, P], [1, H2]]
                            ),
                        )
                        if half == 0 and s == SUB - 1:
                            nc.sync.dma_start(out=wt[:, :], in_=w_t[:, :])
            else:
                nc.sync.dma_start(
                    out=x_in[:, :],
                    in_=bass.AP(x, g * SUB * H, [[NT * H, P], [1, SUB * H]]),
                )
            return x_in

        count_acc = accp.tile([P, E], F32)
        prob_acc = accp.tile([P, E], F32)
        nc.vector.memset(count_acc[:, :], 0.0)
        nc.vector.memset(prob_acc[:, :], 0.0)

        st_probs = accp.tile([P, NT * TOPK], F32)
        st_idx = accp.tile([P, NT * TOPK], U32)
        st_m4 = accp.tile([P, NT * 4], F32)

        def postprocess(g, logits_sb):
            for s in range(SUB):
                tt = g * SUB + s
                lt_ps = ps_t.tile([P, E], F32, tag="tr", name=f"ltp{tt}")
                nc.tensor.transpose(
                    lt_ps[:, :], logits_sb[:, s * P:(s + 1) * P],
                    ident[:E, :E].bitcast(F32),
                )
                lt = small.tile([P, E], F32, tag="lt", name=f"lt{tt}")
                nc.vector.tensor_copy(lt[:, :], lt_ps[:, :])

                maxs = small.tile([P, 8], F32, tag="maxs", name=f"mx{tt}")
                nc.vector.max(out=maxs[:, :], in_=lt[:, :])
                idx8 = small.tile([P, 8], U32, tag="idx8", name=f"ix{tt}")
                nc.vector.max_index(out=idx8[:, :], in_max=maxs[:, :], in_values=lt[:, :])

                nc.vector.tensor_copy(st_m4[:, tt * 4:(tt + 1) * 4], maxs[:, 0:4])
                nc.vector.tensor_copy(st_idx[:, tt * TOPK:(tt + 1) * TOPK], idx8[:, 0:TOPK])

                neg1 = small.tile([P, 1], F32, tag="neg1", name=f"n1{tt}")
                nc.vector.tensor_scalar_mul(neg1[:, :], maxs[:, 0:1], -1.0)

                exp_t = small.tile([P, E], F32, tag="expt", name=f"ex{tt}")
                denom = small.tile([P, 1], F32, tag="denom", name=f"dn{tt}")
                nc.scalar.activation(
                    exp_t[:, :], lt[:, :], mybir.ActivationFunctionType.Exp,
                    bias=neg1[:, :], scale=1.0, accum_out=denom[:, :],
                )
                rden = small.tile([P, 1], F32, tag="rden", name=f"rd{tt}")
                nc.vector.reciprocal(rden[:, :], denom[:, :])
                probs_t = small.tile([P, E], F32, tag="probst", name=f"pb{tt}")
                nc.vector.tensor_scalar_mul(probs_t[:, :], exp_t[:, :], rden[:, :])
                nc.vector.tensor_add(prob_acc[:, :], prob_acc[:, :], probs_t[:, :])

                mask = small.tile([P, E], F32, tag="mask", name=f"mk{tt}")
                nc.vector.tensor_scalar(
                    mask[:, :], lt[:, :], maxs[:, 1:2], None, op0=mybir.AluOpType.is_ge
                )
                nc.vector.tensor_add(count_acc[:, :], count_acc[:, :], mask[:, :])

                # renormalized top-2 probs: p1 = 1/(1+e2), p2 = e2/(1+e2)
                e2 = small.tile([P, 1], F32, tag="e2", name=f"e2{tt}")
                nc.scalar.activation(
                    e2[:, :], maxs[:, 1:2], mybir.ActivationFunctionType.Exp,
                    bias=neg1[:, :], scale=1.0,
                )
                dsum = small.tile([P, 1], F32, tag="dsum", name=f"ds{tt}")
                nc.vector.tensor_scalar_add(dsum[:, :], e2[:, :], 1.0)
                rsum = small.tile([P, 1], F32, tag="rsum", name=f"rs{tt}")
                nc.vector.reciprocal(rsum[:, :], dsum[:, :])
                nc.vector.tensor_copy(st_probs[:, tt * TOPK:tt * TOPK + 1], rsum[:, :])
                nc.vector.tensor_mul(
                    st_probs[:, tt * TOPK + 1:tt * TOPK + 2], e2[:, :], rsum[:, :]
                )

                # flush staged outputs every 4 tiles, via SWDGE (GpSimd is
                # otherwise idle) so SP/ACT stay on the critical path work
                if tt % 4 == 3:
                    lo = tt - 3
                    op = o_probs[:, :].rearrange("(p tt) k -> p (tt k)", p=P)
                    oi = o_idx[:, :].rearrange("(p tt) k -> p (tt k)", p=P)
                    om = o_m4[:, :].rearrange("(p tt) k -> p (tt k)", p=P)
                    nc.gpsimd.dma_start(
                        out=op[:, lo * TOPK:(tt + 1) * TOPK],
                        in_=st_probs[:, lo * TOPK:(tt + 1) * TOPK],
                    )
                    nc.gpsimd.dma_start(
                        out=oi[:, lo * TOPK:(tt + 1) * TOPK],
                        in_=st_idx[:, lo * TOPK:(tt + 1) * TOPK],
                    )
                    nc.gpsimd.dma_start(
                        out=om[:, lo * 4:(tt + 1) * 4],
                        in_=st_m4[:, lo * 4:(tt + 1) * 4],
                    )


        pending = []

        for g in range(NG):
            x_in = load_group(g)

            logits_ps = ps_t.tile([E, G], F32, tag="tr", name=f"lp{g}")
            for c in range(HC):
                if c == 4 and pending:
                    postprocess(*pending.pop(0))
                tr_ps = ps_t.tile([P, G], F32, tag="tr", name=f"tr{g}_{c}")
                for s in range(SUB):
                    # f32r transpose: 1.5 cyc/col vs 2.0 for f32; the FP22
                    # truncation it applies is identical to what the f32r
                    # GEMM read does anyway.
                    nc.tensor.transpose(
                        tr_ps[:, s * P:(s + 1) * P].bitcast(F32R),
                        x_in[:, s * H + c * P:s * H + (c + 1) * P],
                        ident[:, :],
                    )
                xt_sb = xtpool.tile([P, G], F32R, tag="xt", name=f"xt{g}_{c}")
                if c % 2 == 0:
                    nc.scalar.copy(xt_sb[:, :], tr_ps[:, :])
                else:
                    nc.vector.tensor_copy(xt_sb[:, :], tr_ps[:, :])
                nc.tensor.matmul(
                    logits_ps[:, :],
                    wt[:, c * E:(c + 1) * E],
                    xt_sb[:, :],
                    start=(c == 0),
                    stop=(c == HC - 1),
                )

            logits_sb = small.tile([E, G], F32, tag="lsb", name=f"lsb{g}")
            nc.scalar.copy(logits_sb[:, :], logits_ps[:, :])
            pending.append((g, logits_sb))

        for item in pending:
            postprocess(*item)

        nc.gpsimd.dma_start(out=o_stats[:, 0:E], in_=count_acc[:, :])
        nc.gpsimd.dma_start(out=o_stats[:, E:2 * E], in_=prob_acc[:, :])

    nc.finalize()
    return nc


def get_nc():
    if "nc" not in _NC_CACHE:
        _NC_CACHE["nc"] = _build_nc()
    return _NC_CACHE["nc"]


def make_wt_host(Wf):
    """W [E,H] f32 -> [P, HC*E] with wt[p, c*E+e] = trunc22(W[e, c*128+p])."""
    Wt = np.ascontiguousarray(Wf, dtype=np.float32).copy()
    Wt.view(np.uint32)[...] &= np.uint32(0xFFFFFC00)  # FP22 truncation
    return np.ascontiguousarray(
        Wt.reshape(E, HC, P).transpose(2, 1, 0).reshape(P, HC * E)
    )


def _host_postprocess(xt, Wf, probs, idx, m4, counts, psums):
    """Fix tokens whose FP22 top-3 margins are within noise of a tie."""
    thr = 2e-3
    risk = ((m4[:, 0] - m4[:, 1]) < thr) | ((m4[:, 1] - m4[:, 2]) < thr)
    ridx = np.nonzero(risk)[0]
    if ridx.size:
        lg = xt[ridx].astype(np.float64) @ Wf.astype(np.float64).T  # [n, E]
        order = np.argsort(-lg, axis=1, kind="stable")[:, :TOPK].astype(np.int32)
        lsel = np.take_along_axis(lg, order, 1)
        ex = np.exp(lsel - lsel[:, :1])
        pn = ex / ex.sum(1, keepdims=True)
        np.add.at(counts, idx[ridx].reshape(-1), -1.0)
        np.add.at(counts, order.reshape(-1), 1.0)
        idx[ridx] = order
        probs[ridx] = pn.astype(np.float32)
    aux = E * np.sum((counts / B) * (psums / (B * S)))
    return probs, idx, np.float32(aux)


def kernel(x, W):
    from concourse.bass_utils import run_bass_kernel_spmd

    xt = np.ascontiguousarray(np.asarray(x, dtype=np.float32).reshape(B * S, H))
    Wf = np.ascontiguousarray(np.asarray(W, dtype=np.float32))

    nc = get_nc()
    wt_host = make_wt_host(Wf)
    ident_host = np.eye(P, dtype=np.float32)
    in_maps = [
        {"x": xt[i * T:(i + 1) * T], "wt": wt_host, "ident": ident_host}
        for i in range(NCORES)
    ]
    res = run_bass_kernel_spmd(nc, in_maps, core_ids=list(range(NCORES)))

    probs = np.concatenate([r["o_probs"] for r in res.results], axis=0)
    idx = np.concatenate([r["o_idx"] for r in res.results], axis=0).astype(np.int32)
    m4 = np.concatenate([r["o_m4"] for r in res.results], axis=0)
    stats = np.stack([r["o_stats"] for r in res.results])  # [8, P, 2E]
    counts = stats[:, :, :E].sum(axis=(0, 1)).astype(np.float64)
    psums = stats[:, :, E:].sum(axis=(0, 1)).astype(np.float64)

    probs, idx, aux = _host_postprocess(xt, Wf, probs, idx, m4, counts, psums)
    return (
        probs.reshape(B, S, TOPK),
        idx.reshape(B, S, TOPK),
        aux,
    )
